# revision 30
# baseline (speedup 1.0000x reference)
"""CoAttention kernel for 8 TRN2 NeuronCores (Bass/Tile, SPMD).

Problem: B=4 batches x 2 attention directions = 8 independent co-attention
computations -> one per core.  Per core (batch b, direction d):
    Q = wq @ qf + bq        [256, 2304]     (qf = query-side features)
    K = wr @ rf + br        [256, 2304]     (rf = reference-side features)
    S^T = K^T Q             [2304, 2304]    (computed in m-strips of 128)
    attnT = exp(S^T - 40)   (bf16, unnormalized; softmax denom applied at end)
    sums[q] = sum_m attnT[m, q]             (DVE strip-accumulate + one
                                             ones-matmul across partitions)
    out = (rf @ attnT) * (1/sums)           [2048, 2304]
Host assembles: left_att = concat(left, out[b,dir=0]), right_att likewise.

Precision: projection inputs x and weights W in fp16 (10-bit mantissa,
~0.03% RMS -> ~0.5% attn-weight error, well inside the 2e-2 gate); Q/K
stored fp16; attn@V in bf16.  No row-max subtraction: scores are |S| <~ 80,
exp(S-40) stays in fp32/bf16 range; normalization is exact math.

Phase 2 runs in five q-windows (4x512 + 256) software-pipelined
st(w+1) ahead of av(w) so the PE never waits on softmax bookkeeping.

Walrus in this toolchain allows ONE sync-wait per instruction; SafeTileContext
splits multi-wait instructions into standalone wait ops, and splits the
end-of-kernel drain the same way.
"""
import numpy as np
import ml_dtypes

import concourse.bass as bass
import concourse.mybir as mybir
import concourse.tile as tile
from concourse.vector_clock import ScopedClock
from concourse.bass_utils import run_bass_kernel_spmd

B = 4
C = 2048
HW = 48 * 48          # 2304
D = 256
NCORES = 8

CB = C // 128         # 16 c-blocks
DB = D // 128         # 2 d-blocks
MS = HW // 128        # 18 m-strips
# q-windows: 512-wide (one PSUM bank of f32) + 256 tail.  Used both as the
# phase-1 n-chunks and the phase-2 query windows.
WINS = [(0, 512), (512, 512), (1024, 512), (1536, 512), (2048, 256)]
NW = len(WINS)

F32 = mybir.dt.float32
F32R = mybir.dt.float32r
F16 = mybir.dt.float16
BF16 = mybir.dt.bfloat16

# module-level knobs / results (used by test.py)
TRACE = False
LAST_RESULT = None


class SafeTileContext(tile.TileContext):
    """This walrus build allows at most ONE sync wait per instruction.
    Hoist extra waits onto standalone EventSemaphore (wait-only) ops placed
    immediately before, on the same engine queue; same for the final drain."""
    MAX_WAITS = 1

    def _lower_ordered_insts(self, ordered):
        for bname, insts in ordered.items():
            new_list = []
            for inst in insts:
                si = inst.sync_info
                if si is not None and len(si.on_wait) > self.MAX_WAITS:
                    waits = list(si.on_wait)
                    movable = [w for w in waits if w.wait_reg is None]
                    fixed = [w for w in waits if w.wait_reg is not None]
                    keep = fixed + movable[-1:] if movable else fixed
                    hoist = movable[:-1] if movable else []
                    for w in hoist:
                        wi = mybir.InstEventSemaphore(
                            name=self.nc.get_next_instruction_name(),
                            ins=[], outs=[])
                        wi.engine = inst.engine
                        wi.sync_info = mybir.SyncInfo(on_wait=[w], on_update=[])
                        new_list.append(wi)
                    inst.sync_info = mybir.SyncInfo(
                        on_wait=keep, on_update=list(si.on_update))
                new_list.append(inst)
            insts[:] = new_list
        super()._lower_ordered_insts(ordered)

    def _drain_and_barrier(self, tick_clock, wait_clock):
        drain_inst = self.nc.sync.drain()
        wait_clock.add_sem_waits(
            drain_inst.ins, ScopedClock({None: tick_clock.global_clock}))
        si = drain_inst.ins.sync_info
        waits = list(si.on_wait) if si is not None else []
        ups = list(si.on_update) if si is not None else []
        if len(waits) > self.MAX_WAITS:
            drain_inst.ins.sync_info = mybir.SyncInfo(
                on_wait=waits[: self.MAX_WAITS], on_update=ups)
            rest = waits[self.MAX_WAITS:]
            for i in range(0, len(rest), self.MAX_WAITS):
                extra = self.nc.sync.drain()
                extra.ins.sync_info = mybir.SyncInfo(
                    on_wait=rest[i : i + self.MAX_WAITS], on_update=[])
        self.nc.all_engine_barrier()
        assert self.sems is not None
        popped = self.nc._tile_sem_poison_stack.pop()
        assert popped is self._sem_poison
        self.nc.clear_and_free_semaphores(list(self.sems.allocated().values()))
        self.nc.all_engine_barrier()


def build_kernel():
    nc = bass.Bass("TRN2", target_bir_lowering=False, debug=False)

    qf = nc.dram_tensor("qf", [C, HW], F16, kind="ExternalInput")
    rf = nc.dram_tensor("rf", [C, HW], F16, kind="ExternalInput")
    vtb = nc.dram_tensor("vtb", [HW, C], BF16, kind="ExternalInput")
    wqt = nc.dram_tensor("wqt", [C, D], F16, kind="ExternalInput")
    wrt = nc.dram_tensor("wrt", [C, D], F16, kind="ExternalInput")
    bq = nc.dram_tensor("bq", [128, DB], F32, kind="ExternalInput")
    br = nc.dram_tensor("br", [128, DB], F32, kind="ExternalInput")
    out = nc.dram_tensor("out", [C, HW], F32, kind="ExternalOutput")

    with SafeTileContext(nc) as tc:
        with tc.tile_pool(name="persist", bufs=1) as persist, \
             tc.tile_pool(name="dsc", bufs=1, space="DRAM") as dram_scratch:
            # ---- persistent tiles ----
            # Q/K live in per-window tiles so phase-2 reads only dep on the
            # windows they touch (dep tracking is per-tile, not per-region)
            q_w = [persist.tile([128, DB, wsz], F16, name=f"q_w{w}")
                   for w, (_, wsz) in enumerate(WINS)]
            k_w = [persist.tile([128, DB, wsz], F16, name=f"k_w{w}")
                   for w, (_, wsz) in enumerate(WINS)]
            vt = persist.tile([128, MS, C], BF16)       # V^T [m, c]
            bq_t = persist.tile([128, DB], F32)
            br_t = persist.tile([128, DB], F32)
            nbias = persist.tile([128, 1], F32)
            nc.vector.memset(nbias, -40.0)
            ones_f32 = persist.tile([128, 1], F32)
            nc.vector.memset(ones_f32, 1.0)
            ones = ones_f32.bitcast(F32R)
            # warm the ACT exp table so the first real EXP doesn't pay the
            # 1.3us ACT_TABLE_LOAD mid-pipeline
            warm = persist.tile([128, 1], F32)
            nc.scalar.activation(warm, nbias,
                                 mybir.ActivationFunctionType.Exp,
                                 bias=nbias, scale=1.0)

            # ================= phase 1: projections =================
            # DMA granularity: each dma_start costs ~0.65us of serial
            # sync-sequencer dispatch and a single queue moves ~100 GB/s, so
            # keep the total count ~60 while making the first-needed pieces
            # small: per-quad weight tiles, pair-tiles for chunk 0, quads
            # after that.
            with tc.tile_pool(name="wpool", bufs=1) as wpool, \
                 tc.tile_pool(name="xstream", bufs=1) as xstream, \
                 tc.tile_pool(name="p1ps", bufs=2, space="PSUM") as p1ps:
                wqr = wqt.ap().rearrange("(k p) d -> p k d", p=128)
                wrr = wrt.ap().rearrange("(k p) d -> p k d", p=128)
                qfr = qf.ap().rearrange("(k p) n -> p k n", p=128)
                rfr = rf.ap().rearrange("(k p) n -> p k n", p=128)
                # dispatch order puts exactly the first matmul's deps first:
                # W quad 0 and the first x pair, then everything else
                wq_t, wr_t = [], []
                wq_t.append(wpool.tile([128, 4, D], F16, name="wq0"))
                nc.sync.dma_start(out=wq_t[0], in_=wqr[:, 0:4, :])
                x0 = {}
                x0["xq"] = xstream.tile([128, 2, 512], F16, tag="xq2",
                                        bufs=8, name="xq_0_0")
                nc.sync.dma_start(out=x0["xq"], in_=qfr[:, 0:2, 0:512])
                wr_t.append(wpool.tile([128, 4, D], F16, name="wr0"))
                nc.sync.dma_start(out=wr_t[0], in_=wrr[:, 0:4, :])
                x0["xr"] = xstream.tile([128, 2, 512], F16, tag="xr2",
                                        bufs=8, name="xr_0_0")
                nc.sync.dma_start(out=x0["xr"], in_=rfr[:, 0:2, 0:512])
                # interleave the remaining weight quads with the next chunk-0
                # x pairs in consumption order
                for g in range(1, 4):
                    for pre, view in (("xq", qfr), ("xr", rfr)):
                        t = xstream.tile([128, 2, 512], F16, tag=f"{pre}2",
                                         bufs=8, name=f"{pre}_0_{g}")
                        nc.sync.dma_start(
                            out=t, in_=view[:, g * 2:(g + 1) * 2, 0:512])
                        x0[f"{pre}{g}"] = t
                    h = g
                    cs = slice(h * 4, (h + 1) * 4)
                    wq_t.append(wpool.tile([128, 4, D], F16, name=f"wq{h}"))
                    nc.sync.dma_start(out=wq_t[h], in_=wqr[:, cs, :])
                    wr_t.append(wpool.tile([128, 4, D], F16, name=f"wr{h}"))
                    nc.sync.dma_start(out=wr_t[h], in_=wrr[:, cs, :])
                nc.sync.dma_start(out=bq_t, in_=bq.ap())
                nc.sync.dma_start(out=br_t, in_=br.ap())

                for ch, (coff, csz) in enumerate(WINS):
                    qps = [p1ps.tile([128, 512], F32, tag=f"qps{d}",
                                     name=f"qps{d}_{ch}")
                           for d in range(DB)]
                    kps = [p1ps.tile([128, 512], F32, tag=f"kps{d}",
                                     name=f"kps{d}_{ch}")
                           for d in range(DB)]
                    # group size: pairs for the first chunk (fast first
                    # arrival on parallel queues), quads after
                    gsz = 2 if ch == 0 else 4
                    for g in range(CB // gsz):
                        if ch == 0 and g == 0:
                            xq, xr = x0["xq"], x0["xr"]
                        elif ch == 0 and g < 4:
                            xq, xr = x0[f"xq{g}"], x0[f"xr{g}"]
                        else:
                            xq = xstream.tile([128, gsz, 512], F16,
                                              tag=f"xq{gsz}",
                                              bufs=(8 if gsz == 2 else 5),
                                              name=f"xq_{ch}_{g}")
                            nc.sync.dma_start(
                                out=xq[:, :, :csz],
                                in_=qfr[:, g * gsz:(g + 1) * gsz,
                                        coff:coff + csz])
                            xr = xstream.tile([128, gsz, 512], F16,
                                              tag=f"xr{gsz}",
                                              bufs=(8 if gsz == 2 else 5),
                                              name=f"xr_{ch}_{g}")
                            nc.sync.dma_start(
                                out=xr[:, :, :csz],
                                in_=rfr[:, g * gsz:(g + 1) * gsz,
                                        coff:coff + csz])
                        for i in range(gsz):
                            c = g * gsz + i
                            for d in range(DB):
                                nc.tensor.matmul(
                                    qps[d][:, :csz],
                                    wq_t[c // 4][:, c % 4,
                                                 d * 128:(d + 1) * 128],
                                    xq[:, i, :csz],
                                    start=(c == 0), stop=(c == CB - 1))
                                nc.tensor.matmul(
                                    kps[d][:, :csz],
                                    wr_t[c // 4][:, c % 4,
                                                 d * 128:(d + 1) * 128],
                                    xr[:, i, :csz],
                                    start=(c == 0), stop=(c == CB - 1))
                    for d in range(DB):
                        nc.vector.tensor_scalar_add(
                            q_w[ch][:, d, :csz],
                            qps[d][:, :csz], bq_t[:, d:d + 1])
                        nc.vector.tensor_scalar_add(
                            k_w[ch][:, d, :csz],
                            kps[d][:, :csz], br_t[:, d:d + 1])

            # V^T (host-pretransposed bf16): c-chunked loads emitted after
            # phase 1, draining behind the x stream; the first av window only
            # needs the low c-blocks, which land first
            vtr = vtb.ap().rearrange("(s p) c -> p s c", p=128)
            for h in range(4):
                cs = slice(h * 512, (h + 1) * 512)
                nc.sync.dma_start(out=vt[:, :, cs], in_=vtr[:, :, cs])

            # ================= phase 2: S^T, softmax, attn@V ==============
            with tc.tile_pool(name="attn", bufs=2) as attnp, \
                 tc.tile_pool(name="redp", bufs=2) as redp, \
                 tc.tile_pool(name="small", bufs=2) as small, \
                 tc.tile_pool(name="ostage", bufs=6) as ostage, \
                 tc.tile_pool(name="sps", bufs=2, space="PSUM") as spsp, \
                 tc.tile_pool(name="sums", bufs=1, space="PSUM") as sumsp, \
                 tc.tile_pool(name="ops", bufs=5, space="PSUM") as opsp:
                attn_ts = {}
                reds = {}
                bcs = {}

                def st_new(w):
                    attn_ts[w] = attnp.tile([128, MS, 512], BF16, tag="attnT",
                                            name=f"attnT_{w}")
                    reds[w] = redp.tile([128, 512], F32R, tag="red",
                                        name=f"red_{w}")

                def st_strips(w, ms):
                    woff, wsz = WINS[w]
                    attn_t = attn_ts[w]
                    red = reds[w]
                    for m in ms:
                        kw, koff = divmod(m * 128, 512)
                        sps = spsp.tile([128, 512], F32, tag="sps")
                        for d in range(DB):
                            nc.tensor.matmul(
                                sps[:, :wsz],
                                k_w[kw][:, d, koff:koff + 128],
                                q_w[w][:, d, :wsz],
                                start=(d == 0), stop=(d == DB - 1))
                        nc.scalar.activation(
                            attn_t[:, m, :wsz], sps[:, :wsz],
                            mybir.ActivationFunctionType.Exp,
                            bias=nbias, scale=1.0)
                        # strip-accumulate the softmax denominator on DVE
                        if m == 1:
                            nc.vector.scalar_tensor_tensor(
                                red[:, :wsz], attn_t[:, 0, :wsz], 0.0,
                                attn_t[:, 1, :wsz],
                                op0=mybir.AluOpType.add,
                                op1=mybir.AluOpType.add)
                        elif m >= 2:
                            nc.vector.scalar_tensor_tensor(
                                red[:, :wsz], attn_t[:, m, :wsz], 0.0,
                                red[:, :wsz],
                                op0=mybir.AluOpType.add,
                                op1=mybir.AluOpType.add)

                def fin_phase(w):
                    # softmax denominator: single ones-matmul across
                    # partitions (f32r, full PE rate), fast reciprocal on the
                    # [1, w] row, then gpsimd partition-broadcast — short
                    # SBUF-only chain, no DRAM round trip
                    woff, wsz = WINS[w]
                    red = reds.pop(w)
                    sums_ps = sumsp.tile([1, 512], F32, tag="sums",
                                         name=f"sums_{w}")
                    nc.tensor.matmul(sums_ps[:, :wsz], ones, red[:, :wsz],
                                     start=True, stop=True)
                    sums_sb = small.tile([1, 512], F32, tag="sums_sb",
                                         name=f"sums_sb_{w}")
                    nc.scalar.copy(sums_sb[:, :wsz], sums_ps[:, :wsz])
                    invs = small.tile([1, 512], F32, tag="invs",
                                      name=f"invs_{w}")
                    nc.vector.reciprocal(invs[:, :wsz], sums_sb[:, :wsz])
                    invs_dram = dram_scratch.tile([1, wsz], F32,
                                                  tag=f"invd{w}",
                                                  name=f"invd_{w}")
                    nc.sync.dma_start(out=invs_dram, in_=invs[:, :wsz])
                    bc = small.tile([128, 512], F32, tag="bc", name=f"bc_{w}")
                    bcs[w] = bc
                    nc.sync.dma_start(out=bc[:, :wsz],
                                      in_=invs_dram.partition_broadcast(128))

                def av_phase(w, nxt, first=False):
                    # attn@V for window w; the next window's S^T strips are
                    # interleaved between c-blocks so their ACT-exp latency
                    # hides behind av matmuls instead of pacing the PE.
                    # fin(w) must come before any stt of window w on the DVE
                    # queue (the stts wait on bc, which the fin's reciprocal
                    # produces — strict FIFO would deadlock otherwise): fin(0)
                    # sits at av0 cb0; fin(w+1) at av(w) cb14, by which point
                    # red(w+1) is complete and the ~13us bc chain still
                    # finishes before av(w+1)'s stts need it.
                    woff, wsz = WINS[w]
                    attn_t = attn_ts.pop(w)
                    for cb in range(CB):
                        ops = opsp.tile([128, 512], F32, tag="ops")
                        for m in range(MS):
                            nc.tensor.matmul(
                                ops[:, :wsz],
                                vt[:, m, cb * 128:(cb + 1) * 128],
                                attn_t[:, m, :wsz],
                                start=(m == 0), stop=(m == MS - 1))
                        if cb == 0 and first:
                            fin_phase(w)
                        if cb == 0:
                            bc = bcs.pop(w)
                        o_sb = ostage.tile([128, 512], F32, tag="osb",
                                           name=f"osb_{w}_{cb}")
                        nc.vector.scalar_tensor_tensor(
                            o_sb[:, :wsz], ops[:, :wsz], 0.0,
                            bc[:, :wsz],
                            op0=mybir.AluOpType.add,
                            op1=mybir.AluOpType.mult)
                        nc.gpsimd.dma_start(
                            out=out.ap()[cb * 128:(cb + 1) * 128,
                                         woff:woff + wsz],
                            in_=o_sb[:, :wsz])
                        if nxt is not None and cb < 13:
                            # pack the next window's strips into the first 13
                            # c-blocks so its last exp lands well before the
                            # window boundary
                            st_strips(nxt, range((MS * cb) // 13,
                                                 (MS * (cb + 1)) // 13))
                        if nxt is not None and cb == 14:
                            fin_phase(nxt)

                # process the short 256 window first: its standalone st fill
                # bubble is ACT-paced, so the narrow window halves the bubble
                order = [4, 0, 1, 2, 3]
                st_new(order[0])
                st_strips(order[0], range(MS))
                for i, w in enumerate(order):
                    nxt = order[i + 1] if i + 1 < NW else None
                    if nxt is not None:
                        st_new(nxt)
                    av_phase(w, nxt, first=(i == 0))
    return nc


def kernel(left_features, right_features, wq, bq, wr, br):
    global LAST_RESULT
    left = np.asarray(left_features, dtype=np.float32)
    right = np.asarray(right_features, dtype=np.float32)
    wq = np.asarray(wq, dtype=np.float32)
    wr = np.asarray(wr, dtype=np.float32)
    bq = np.asarray(bq, dtype=np.float32)
    br = np.asarray(br, dtype=np.float32)

    lf = left.reshape(B, C, HW)
    rg = right.reshape(B, C, HW)
    lf_h = lf.astype(np.float16)
    rg_h = rg.astype(np.float16)
    wqt = np.ascontiguousarray(wq.T).astype(np.float16)   # [C, D]
    wrt = np.ascontiguousarray(wr.T).astype(np.float16)
    bq_t = np.ascontiguousarray(bq.reshape(DB, 128).T)    # [128, DB]
    br_t = np.ascontiguousarray(br.reshape(DB, 128).T)

    nc = build_kernel()
    in_maps = []
    for core in range(NCORES):
        b, d = core // 2, core % 2
        qf_c = lf_h[b] if d == 0 else rg_h[b]
        rf_c = rg_h[b] if d == 0 else lf_h[b]
        rf_full = rg[b] if d == 0 else lf[b]
        in_maps.append({
            "qf": np.ascontiguousarray(qf_c),
            "rf": np.ascontiguousarray(rf_c),
            "vtb": np.ascontiguousarray(rf_full.T.astype(ml_dtypes.bfloat16)),
            "wqt": wqt, "wrt": wrt, "bq": bq_t, "br": br_t,
        })
    res = run_bass_kernel_spmd(nc, in_maps, core_ids=list(range(NCORES)),
                               trace=TRACE)
    LAST_RESULT = res

    weighted = np.stack([res.results[core]["out"] for core in range(NCORES)])
    weighted = weighted.reshape(B, 2, C, 48, 48)
    left_att = np.concatenate([left, weighted[:, 0]], axis=1)
    right_att = np.concatenate([right, weighted[:, 1]], axis=1)
    return (left_att, right_att)
